# revision 37
# baseline (speedup 1.0000x reference)
"""Block-diagonal causal GQA attention with RoPE, sharded over 8 TRN2 cores.

Problem (hardcoded from the spec):
  x [4096, 4096], wq [4096, 4096] (32 q heads x 128), wk/wv [4096, 1024]
  (8 kv heads), wo [4096, 4096], freqs_cos/sin [4096, 64], block_size 1024.
  4 independent causal blocks of 1024 tokens.

Sharding: 8 cores = 4 sequence blocks x 2 head-groups.  Core (b, g)
computes block b for q-heads [16g, 16g+16) (kv heads [4g, 4g+4)) and the
partial output projection through the matching rows of wo.  The host sums
the two head-group partials per block and concatenates the blocks.

Device layout notes:
  - all inputs are fed as bf16 (host-side cast); PSUM accumulation and the
    final output stay fp32.  End-to-end quantization error ~5e-3 rel.
  - x block is fed pre-transposed (xbT [4096, 1024]) so the DIM contraction
    sits on SBUF partitions for the QKV projections.
  - wq/wk columns are de-interleaved per head on the host (even head-dims
    then odd head-dims) so RoPE's pair rotation becomes a [0:64]/[64:128]
    partition-half operation on the Q^T/K^T layout.
  - attention runs in the transposed-score layout S^T [j, i]: scores come
    out of the PE as S^T tiles, the causal mask is accumulated into the
    diagonal blocks by a tiny PE matmul (maskT @ identity), the softmax
    denominator is a ones-vector matmul, and P^T feeds the PV and WO
    matmuls directly (no transposes).
  - the softmax reciprocal is split across engines (chunk 0: DVE
    reciprocal; chunk 1: ACT exp(-ln(x))) and its K=1 broadcast matmul +
    normalize are deferred one head, so the PE never waits on it.
  - wo tiles for the first output chunks prefetch during attention (the
    DMA rings are idle there).
"""

import numpy as np
from contextlib import ExitStack

import concourse.bass as bass
import concourse.tile as tile
import concourse.mybir as mybir
from concourse import bass_utils
import ml_dtypes

F32 = mybir.dt.float32
BF16 = mybir.dt.bfloat16
F32R = mybir.dt.float32r

# -- full-problem constants ---------------------------------------------------
DIM = 4096
BLOCK = 1024
D = 128            # head dim
HQ = 16            # q heads per core
HKV = 4            # kv heads per core
N_CORES = 8
NEG = -1.0e9


def _trim_dma_waits(nc):
    """Drop DMA semaphore waits that are transitively guaranteed.

    The DGE descriptor path supports only 2 sync-wait commands per DMA,
    but Tile's wait emission is not transitively minimal.  We compute,
    for every instruction, a conservative "floor": the semaphore values
    guaranteed to have been reached by the time it completes (its own
    waits, the floors of the instructions those waits observe, the
    floors of its sync dependencies, plus in-order completion along each
    semaphore's single FIFO ring).  A wait on a DMA is dead if the
    floors implied by its remaining waits already cover it.
    """
    import bass_rust

    insts = []
    for blk in nc.m.functions[0].blocks:
        insts.extend(blk.instructions)

    floors: dict[str, dict[int, int]] = {}     # inst name -> {sem id: value}
    chain: dict[int, list[tuple[int, str]]] = {}  # sem id -> [(post_val, name)]
    cum: dict[int, int] = {}

    def sem_floor(sem_id, v):
        """Floor implied by observing sem_id >= v (completion of the
        instruction whose update reached v, FIFO within a sem)."""
        lst = chain.get(sem_id)
        if not lst:
            return None
        # smallest post_val >= v
        import bisect
        idx = bisect.bisect_left(lst, (v, ""))
        if idx == len(lst):
            return None
        return floors.get(lst[idx][1])

    def merge(dst, src):
        if not src:
            return
        for k, v in src.items():
            if dst.get(k, -1) < v:
                dst[k] = v

    for ins in insts:
        si = ins.sync_info
        fl: dict[int, int] = {}
        if si is not None:
            for w in si.on_wait:
                if w.wait_mode != "sem-ge-imm" or w.wait_value is None:
                    continue
                if fl.get(w.id, -1) < w.wait_value:
                    fl[w.id] = w.wait_value
                merge(fl, sem_floor(w.id, w.wait_value))
        try:
            for dn in ins.sync_dependency_names():
                merge(fl, floors.get(dn))
        except TypeError:
            pass
        if si is not None:
            for u in si.on_update:
                if u.update_mode not in ("sem-add-imm", "sem-inc") \
                        or u.update_value is None:
                    continue
                post = cum.get(u.id, 0) + u.update_value
                cum[u.id] = post
                lst = chain.setdefault(u.id, [])
                # in-order completion per sem ring: inherit previous floor
                if lst:
                    merge(fl, floors.get(lst[-1][1]))
                if fl.get(u.id, -1) < post:
                    fl[u.id] = post
                lst.append((post, ins.name))
        floors[ins.name] = fl

    for ins in insts:
        if not isinstance(ins, mybir.InstDMACopy):
            continue
        si = ins.sync_info
        if si is None:
            continue
        waits = list(si.on_wait)
        changed = True
        while len(waits) > 1 and changed:
            changed = False
            for i, w in enumerate(waits):
                if w.wait_mode != "sem-ge-imm" or w.wait_value is None:
                    continue
                implied: dict[int, int] = {}
                for j, w2 in enumerate(waits):
                    if j == i or w2.wait_mode != "sem-ge-imm":
                        continue
                    merge(implied, sem_floor(w2.id, w2.wait_value))
                if implied.get(w.id, -1) >= w.wait_value:
                    waits.pop(i)
                    changed = True
                    break
        if len(waits) != len(si.on_wait):
            ins.sync_info = bass_rust.SyncInfo(
                on_wait=waits, on_update=list(si.on_update))


def _split_waits_json(bir):
    """Split multi-wait instructions at the BIR level.

    walrus' setupSyncWait budget: one wait of any value, or two waits
    whose values both fit a one-byte command.  Excess waits move onto
    standalone EventSemaphore instructions inserted directly before the
    instruction on the same engine — engines execute their stream in
    order, so a prefix wait is semantically identical to an attached
    one.  (DMAs are enqueued by their issuing engine in stream order,
    so the same argument holds for the enqueue.)
    """
    nid = 0
    for fn in bir["functions"]:
        for blk in fn["blocks"]:
            out = []
            for ins in blk["instructions"]:
                si = ins.get("sync_info")
                waits = (si or {}).get("on_wait") or []
                if len(waits) > 1:
                    waits = sorted(
                        waits, key=lambda w: -(w.get("wait_value") or 0))
                    for w in waits[1:]:
                        nid += 1
                        out.append({
                            "debug": ins.get("debug"),
                            "engine": ins["engine"],
                            "ins": [],
                            "outs": [],
                            "name": f"{ins['name']}-w{nid}",
                            "opcode": "EventSemaphore",
                            "sync_info": {"on_update": [], "on_wait": [w]},
                        })
                    si["on_wait"] = waits[:1]
                out.append(ins)
            blk["instructions"] = out
    return bir


def build_kernel(dim=DIM, block=BLOCK, hq=HQ, hkv=HKV):
    """Emit the per-core Bass program (SPMD: same program on all cores)."""
    rep = hq // hkv
    KC = dim // 128          # contraction chunks
    assert KC % 2 == 0
    KH = KC // 2             # chunks per half
    W = min(512, block)      # psum free width
    CH = block // W          # i-chunks per block
    NI = block // 128        # j-tiles per block
    assert NI <= 8
    HSET = max(1, 8 // CH)   # q heads per psum-set
    HALF_D = D // 2
    SCALE = float(1.0 / np.sqrt(D))
    assert hkv * D <= 512
    WO_PRE = 2               # wo col-chunks prefetched during attention

    nc = bass.Bass("TRN2", target_bir_lowering=False, debug=False)

    xbT = nc.dram_tensor("xbT", [dim, block], BF16, kind="ExternalInput").ap()
    wq = nc.dram_tensor("wq", [dim, hq * D], BF16, kind="ExternalInput").ap()
    wk = nc.dram_tensor("wk", [dim, hkv * D], BF16, kind="ExternalInput").ap()
    wv = nc.dram_tensor("wv", [dim, hkv * D], BF16, kind="ExternalInput").ap()
    wo = nc.dram_tensor("wo", [hq * D, dim], BF16, kind="ExternalInput").ap()
    # cos2: [cos; cos] stacked to 128 partitions; sin2: [-sin; +sin]
    cos2 = nc.dram_tensor("cos2", [D, block], BF16, kind="ExternalInput").ap()
    sin2 = nc.dram_tensor("sin2", [D, block], BF16, kind="ExternalInput").ap()
    out = nc.dram_tensor("out", [block, dim], F32, kind="ExternalOutput").ap()

    def mm(out_ap, lhsT, rhs, **kw):
        nc.tensor.matmul(out_ap, lhsT, rhs, **kw)

    # expS free-dim layout: j-tile t occupies [offs[t], offs[t] + block - 128 t)
    offs = []
    o = 0
    for t in range(NI):
        offs.append(o)
        o += block - t * 128
    EW = o

    with tile.TileContext(nc) as tc, ExitStack() as ctx:
        const = ctx.enter_context(tc.tile_pool(name="const", bufs=1))
        # memset only emits fp32 patterns: write two packed bf16 1.0s.
        # 32 columns so the denominator matmuls fill psum partitions 0..31 /
        # 32..63 with replicated sums -- every lane the reciprocal pass
        # reads is then matmul-written (no stale-psum junk lanes).
        ones = const.tile([128, 32], BF16)         # denominator lhsT
        ONES2 = float(np.frombuffer(
            np.array([0x3F803F80], dtype=np.uint32).tobytes(),
            dtype=np.float32)[0])
        nc.gpsimd.memset(ones[:].bitcast(F32), ONES2)
        # bc lhsT (parts 0/32 used): -1 so the negated approx reciprocal
        # (see recip_both) broadcasts with the right sign
        ones_row = const.tile([128, 128], F32R)
        nc.gpsimd.memset(ones_row[:].bitcast(F32), -1.0)


        acc_pool = ctx.enter_context(tc.tile_pool(name="accs", bufs=1))
        if True:
            # persistent SBUF accumulators, one big tile each, sliced per head
            qTa = acc_pool.tile([128, hq * block], BF16)     # per head: [d, i]
            kTa = acc_pool.tile([128, hkv * block], BF16)    # per kv head: [d, j]
            va = acc_pool.tile([128, NI * hkv * D], BF16)    # per j-tile: [j, hkv*D]
            oTall = acc_pool.tile([128, hq * block], BF16)   # per head: [d, i]

            # ---- QKV projections, two k-halves so SBUF holds half of xbT.
            # half 1 runs K-heads first and emits each head's RoPE right
            # after its final accumulation, so the DVE rotation work (and
            # the swap DMAs) overlap the remaining Q/V matmuls and the
            # first attention heads are ready as early as possible. ----
            with tc.tile_pool(name="ropep", bufs=3) as rp, \
                 tc.tile_pool(name="xbp", bufs=KH) as xb_pool, \
                 tc.tile_pool(name="wsp", bufs=6) as ws_pool, \
                 tc.tile_pool(name="qkvps", bufs=8, space="PSUM") as ps_pool:

                cos_sb = rp.tile([D, block], BF16, name="cos_sb", tag="cos",
                                 bufs=1)
                sin_sb = rp.tile([D, block], BF16, name="sin_sb", tag="sin",
                                 bufs=1)
                cs_loaded = False

                def acc_store(dst, ps, half):
                    if half == 0:
                        nc.scalar.copy(dst, ps)
                    else:
                        nc.vector.tensor_add(dst, dst, ps)

                def rope(base):
                    # Layout per head: partitions [0:64] = even head-dims,
                    # [64:128] = odd.  rope(x) = x*cos2 + swap(x)*sin2 where
                    # swap exchanges the halves (SBUF->SBUF DMA) and
                    # sin2 = [-sin; +sin], cos2 = [cos; cos].
                    sw = rp.tile([D, block], BF16, name="sw", tag="sw")
                    # swap DMAs ride the ACT ring, off the sync load ring
                    nc.scalar.dma_start(sw[0:HALF_D, :], base[HALF_D:D, :])
                    nc.scalar.dma_start(sw[HALF_D:D, :], base[0:HALF_D, :])
                    tmp = rp.tile([D, block], BF16, name="rtmp", tag="rtmp")
                    nc.vector.tensor_mul(tmp[:], sw[:], sin_sb[:])
                    nc.vector.tensor_mul(base, base, cos_sb[:])
                    nc.vector.tensor_add(base, base, tmp[:])

                def q_heads(half, xb, load_xb):
                    for hs in range(0, hq, HSET):
                        nh = min(HSET, hq - hs)
                        ps = [ps_pool.tile([128, CH * W], F32, name="qps",
                                           tag="ps2", bufs=4)
                              for _ in range(nh)]
                        for kk in range(KH):
                            k = half * KH + kk
                            if load_xb and hs == 0:
                                xt = xb_pool.tile([128, block], BF16,
                                                  name="xbt", tag="xb")
                                nc.sync.dma_start(
                                    xt[:], xbT[k * 128:(k + 1) * 128, :])
                                xb.append(xt)
                            wt = ws_pool.tile([128, HSET * D], BF16,
                                              name="wqs", tag="ws")
                            nc.sync.dma_start(
                                wt[:, :nh * D],
                                wq[k * 128:(k + 1) * 128, hs * D:(hs + nh) * D])
                            for hl in range(nh):
                                for c in range(CH):
                                    mm(ps[hl][:, c * W:(c + 1) * W],
                                       wt[:, hl * D:(hl + 1) * D],
                                       xb[kk][:, c * W:(c + 1) * W],
                                       start=(kk == 0), stop=(kk == KH - 1))
                        for hl in range(nh):
                            h = hs + hl
                            acc_store(qTa[:, h * block:(h + 1) * block],
                                      ps[hl][:], half)
                            if half == 1:
                                rope(qTa[:, h * block:(h + 1) * block])

                def k_heads(half, xb, load_xb):
                    for hs in range(0, hkv, HSET):
                        nh = min(HSET, hkv - hs)
                        ps = [ps_pool.tile([128, CH * W], F32, name="kps",
                                           tag="ps2", bufs=4)
                              for _ in range(nh)]
                        for kk in range(KH):
                            k = half * KH + kk
                            if load_xb and hs == 0:
                                xt = xb_pool.tile([128, block], BF16,
                                                  name="xbt", tag="xb")
                                nc.sync.dma_start(
                                    xt[:], xbT[k * 128:(k + 1) * 128, :])
                                xb.append(xt)
                            wt = ws_pool.tile([128, HSET * D], BF16,
                                              name="wks", tag="ws")
                            nc.sync.dma_start(
                                wt[:, :nh * D],
                                wk[k * 128:(k + 1) * 128, hs * D:(hs + nh) * D])
                            for hl in range(nh):
                                for c in range(CH):
                                    mm(ps[hl][:, c * W:(c + 1) * W],
                                       wt[:, hl * D:(hl + 1) * D],
                                       xb[kk][:, c * W:(c + 1) * W],
                                       start=(kk == 0), stop=(kk == KH - 1))
                        for hl in range(nh):
                            h = hs + hl
                            acc_store(kTa[:, h * block:(h + 1) * block],
                                      ps[hl][:], half)
                            if half == 1:
                                rope(kTa[:, h * block:(h + 1) * block])

                def v_heads(half, xb):
                    VP = hkv * D
                    vpair = 2 if (NI % 2 == 0 and VP * 4 >= 2048) else 1
                    for ts in range(0, NI, 8):
                        nt = min(8, NI - ts)
                        ps = [ps_pool.tile([128, vpair * VP], F32, name="vps",
                                           tag="ps2" if vpair == 2 else "ps",
                                           bufs=4)
                              for _ in range(nt // vpair)]
                        for kk in range(KH):
                            wt = ws_pool.tile([128, HSET * D], BF16,
                                              name="wvs", tag="ws")
                            k = half * KH + kk
                            nc.sync.dma_start(
                                wt[:, :VP], wv[k * 128:(k + 1) * 128, :])
                            for tl in range(nt):
                                tj = ts + tl
                                o = (tl % vpair) * VP
                                mm(ps[tl // vpair][:, o:o + VP],
                                   xb[kk][:, tj * 128:(tj + 1) * 128],
                                   wt[:, :VP],
                                   start=(kk == 0), stop=(kk == KH - 1))
                        for tl in range(0, nt, vpair):
                            tj = ts + tl
                            dst = va[:, tj * VP:(tj + vpair) * VP]
                            acc_store(dst, ps[tl // vpair][:], half)

                # half 0: Q, K, V (xb loads interleaved into the Q loop so
                # the first weight tile isn't stuck behind 8 MB of x).
                # cos/sin loads are enqueued after the first head-set's
                # loads: the DGE ring is FIFO and rope needs them late.
                xb0 = []
                q_heads(0, xb0, load_xb=True)
                if not cs_loaded:
                    nc.sync.dma_start(cos_sb[:], cos2)
                    nc.sync.dma_start(sin_sb[:], sin2)
                    cs_loaded = True
                k_heads(0, xb0, load_xb=False)
                v_heads(0, xb0)
                # half 1: K first (rope k early), then Q (rope per head),
                # then V.
                # half 1: K first (rope k early), V, then Q last so the
                # first q heads' ropes land well before attention starts
                xb1 = []
                k_heads(1, xb1, load_xb=True)
                v_heads(1, xb1)
                q_heads(1, xb1, load_xb=False)

            # wo prefetch pool: opened before the attention pools so the
            # first WO_PRE column-chunks stream in while attention runs
            # (the DMA rings are otherwise idle there).
            wo_pool = ctx.enter_context(
                tc.tile_pool(name="wow", bufs=2 * hq, side="right"))
            wo_tiles = {}
            for nch in range(WO_PRE):
                wts = []
                for h in range(hq):
                    wt = wo_pool.tile([128, W], BF16, name="wot", tag="wot")
                    nc.sync.dma_start(
                        wt[:], wo[h * D:(h + 1) * D, nch * W:(nch + 1) * W])
                    wts.append(wt)
                wo_tiles[nch] = wts

            # ---- attention, 16 heads, software-pipelined ----
            # Per head h the PE stream is:
            #   bc(h-1,c0) | scores t0..t7 (+mask) with bc(h-1,c1) after t0
            #   | denominator ones-mms | PV
            # and the DVE/ACT work (exp, reciprocal, normalize) always
            # references data from >= 1 head earlier, so the PE never
            # stalls and HAM stays warm.
            # Scores psum groups pack the causal trapezoid's 8 j-tiles into
            # 5 contiguous regions -- (t0) (t1|t7|t1) (t2|t6|t2) (t3|t5|t3)
            # (t4) -- so softmax exp needs only 5 ACT instructions per head
            # (ACT's ~0.3us fixed cost per instruction made 8+4 the
            # bottleneck).  Every segment stays inside one psum bank.
            assert NI == 8 and CH == 2
            GROUPS = [[(0, 0), (0, 1)],
                      [(1, 0), (7, 1), (1, 1)],
                      [(2, 0), (6, 1), (2, 1)],
                      [(3, 0), (5, 1), (3, 1)],
                      [(4, 1)]]

            def segw(t, c):
                return (c + 1) * W - max(t * 128, c * W)

            seg_off = {}
            gbase = []
            o = 0
            for grp in GROUPS:
                gbase.append(o)
                for t, c in grp:
                    seg_off[(t, c)] = o
                    o += segw(t, c)
            assert o == EW

            with tc.tile_pool(name="attsb", bufs=2) as att_sb, \
                 tc.tile_pool(name="stps", bufs=2, space="PSUM") as st_ps, \
                 tc.tile_pool(name="st4ps", bufs=1, space="PSUM") as st4_ps, \
                 tc.tile_pool(name="sumps", bufs=1, space="PSUM") as sum_ps, \
                 tc.tile_pool(name="pvps", bufs=1, space="PSUM") as pv_ps:

                def denom_c(h, expS, sp, c):
                    # ones^T @ expS^T for chunk c; lands on psum partition
                    # 32c so both chunks share one psum bank
                    live = [t for t in range(NI) if t * 128 < (c + 1) * W]
                    for idx, t in enumerate(live):
                        s0 = max(t * 128, c * W)
                        w = (c + 1) * W - s0
                        mm(sp[32 * c:32 * c + 32, s0 - c * W:s0 - c * W + w],
                           ones[:, 0:32], expS[:, seg_off[(t, c)]:
                                               seg_off[(t, c)] + w],
                           start=(idx == 0), stop=(idx == len(live) - 1))

                # 1/den for both chunks in one 3-instruction DVE pass over
                # partitions 0..32 (lanes 1..31 compute junk that nothing
                # reads): bitwise-NOT seed + one Newton step, ~0.2% error.
                # Produces -1/den; the -1 ones_row broadcast flips it back.
                # Replaces the 3.3us iterative divide + ACT ln/exp pair.
                RC_C0, RC_C1 = -0.23549792, 2.0017324

                def recip_both(h, sp, rc):
                    # nt = ~bits(den); u = den*nt; v = c0*u - c1;
                    # rc = (v*c0)*nt = -1/den (to ~0.2%)
                    nt = att_sb.tile([64, W], F32, name="rnt", tag="rnt")
                    u = att_sb.tile([64, W], F32, name="rtu", tag="rtu")
                    nc.vector.tensor_scalar(
                        nt[:].bitcast(mybir.dt.int32), 
                        sp[0:64, :W].bitcast(mybir.dt.int32), 0, None,
                        mybir.AluOpType.bitwise_not)
                    nc.vector.tensor_mul(u[:, :W], sp[0:64, :W], nt[:, :W])
                    nc.vector.tensor_scalar(
                        u[:, :W], u[:, :W], RC_C0, RC_C1,
                        mybir.AluOpType.mult, mybir.AluOpType.subtract)
                    with nc.allow_low_precision("f32r matmul operand"):
                        nc.vector.scalar_tensor_tensor(
                            rc[0:64, :W], u[:, :W], RC_C0, nt[:, :W],
                            mybir.AluOpType.mult, mybir.AluOpType.mult)

                def pv_mms(h, expS):
                    kv = h // rep
                    pv = pv_ps.tile([128, CH * W], F32, name="pv", tag="pv")
                    for c in range(CH):
                        live = [t for t in range(NI) if t * 128 < (c + 1) * W]
                        for idx, t in enumerate(live):
                            s0 = max(t * 128, c * W)
                            w = (c + 1) * W - s0
                            mm(pv[:, s0:s0 + w],
                               va[:, t * hkv * D + kv * D:
                                  t * hkv * D + (kv + 1) * D],
                               expS[:, seg_off[(t, c)]:seg_off[(t, c)] + w],
                               start=(idx == 0), stop=(idx == len(live) - 1))
                    return pv

                def bc_norm(h, rc, pv, c):
                    # broadcast 1/den over partitions (K=1 matmul into the
                    # st4 psum bank, idle between heads), stage to SBUF on
                    # DVE (DVE reads only one PSUM operand), then normalize
                    bc = st4_ps.tile([128, W], F32, name="bc", tag="st4")
                    mm(bc[:], ones_row[32 * c:32 * c + 1, :],
                       rc[32 * c:32 * c + 1, :W], start=True, stop=True)
                    rb = att_sb.tile([128, W], F32, name="rb", tag="rb")
                    nc.vector.tensor_copy(rb[:], bc[:])
                    nc.vector.tensor_mul(
                        oTall[:, h * block + c * W:h * block + (c + 1) * W],
                        pv[:, c * W:(c + 1) * W], rb[:])

                def emit_scores(h, expS, gis):
                    kv = h // rep
                    qT = qTa[:, h * block:(h + 1) * block]
                    kT = kTa[:, kv * block:(kv + 1) * block]
                    for gi in gis:
                        grp = GROUPS[gi]
                        gw = sum(segw(t, c) for t, c in grp)
                        stt = (st_ps.tile([128, CH * W], F32, name="st",
                                          tag="st") if gi < 4 else
                               st4_ps.tile([128, W], F32, name="st4",
                                           tag="st4"))
                        for t, c in grp:
                            s0 = max(t * 128, c * W)
                            w = (c + 1) * W - s0
                            lo = seg_off[(t, c)] - gbase[gi]
                            mm(stt[:, lo:lo + w],
                               kT[:, t * 128:t * 128 + 128],
                               qT[:, s0:s0 + w], start=True, stop=True)
                        nc.scalar.activation(
                            expS[:, gbase[gi]:gbase[gi] + gw], stt[:, 0:gw],
                            mybir.ActivationFunctionType.Exp, scale=SCALE)
                        # causal mask: zero exp of the upper triangle of
                        # each j-tile's diagonal 128x128 block (keep where
                        # i_local >= j_local) on GPSIMD
                        for t, c in grp:
                            if c != (t * 128) // W:
                                continue
                            nc.gpsimd.affine_select(
                                out=expS[:, seg_off[(t, c)]:
                                         seg_off[(t, c)] + 128],
                                in_=expS[:, seg_off[(t, c)]:
                                         seg_off[(t, c)] + 128],
                                compare_op=mybir.AluOpType.is_ge,
                                fill=0.0, base=0, pattern=[[1, 128]],
                                channel_multiplier=-1,
                            )

                # Three-stage software pipeline over heads: iteration k runs
                # scores+exp of head k on PE+ACT, denominators+reciprocal+PV
                # of head k-1 (their exps completed last iteration), and the
                # broadcast+normalize of head k-2.  The head k-1/k-2 work is
                # interleaved BETWEEN head k's score groups so each st psum
                # slot gets ~1.5us of slack before its reuse and the PE
                # stream never drains (draining also re-throttles HAM).
                S = {}          # live per-head state: expS/sp/rc/pv
                for k in range(hq + 1):
                    live = k < hq
                    if live:
                        S[k] = {"expS": att_sb.tile([128, EW], BF16,
                                                    name="expS", tag="expS"),
                                "rc": att_sb.tile([64, W], F32R,
                                                  name="rc", tag="rc")}
                        emit_scores(k, S[k]["expS"], [0])
                    if k >= 2:
                        bc_norm(k - 2, S[k - 2]["rc"], S[k - 2]["pv"], 0)
                    if live:
                        emit_scores(k, S[k]["expS"], [1])
                    if k >= 1:
                        S[k - 1]["sp"] = sum_ps.tile([128, W], F32,
                                                     name="sump", tag="sump")
                        denom_c(k - 1, S[k - 1]["expS"], S[k - 1]["sp"], 0)
                    if live:
                        emit_scores(k, S[k]["expS"], [2])
                    if k >= 2:
                        bc_norm(k - 2, S[k - 2]["rc"], S[k - 2]["pv"], 1)
                        del S[k - 2]
                    if live:
                        emit_scores(k, S[k]["expS"], [3])
                    if k >= 1:
                        denom_c(k - 1, S[k - 1]["expS"], S[k - 1]["sp"], 1)
                        recip_both(k - 1, S[k - 1]["sp"], S[k - 1]["rc"])
                    if live:
                        emit_scores(k, S[k]["expS"], [4])
                    if k >= 1:
                        S[k - 1]["pv"] = pv_mms(k - 1, S[k - 1]["expS"])
                bc_norm(hq - 1, S[hq - 1]["rc"], S[hq - 1]["pv"], 0)
                bc_norm(hq - 1, S[hq - 1]["rc"], S[hq - 1]["pv"], 1)

        # ---- output projection: out = O @ wo_g ----
        with tc.tile_pool(name="woout", bufs=4) as out_pool, \
             tc.tile_pool(name="wops", bufs=4, space="PSUM") as wo_ps:
            for nch in range(dim // W):
                if nch in wo_tiles:
                    wts = wo_tiles[nch]
                else:
                    wts = []
                    for h in range(hq):
                        wt = wo_pool.tile([128, W], BF16, name="wot",
                                          tag="wot")
                        nc.sync.dma_start(
                            wt[:],
                            wo[h * D:(h + 1) * D, nch * W:(nch + 1) * W])
                        wts.append(wt)
                for it in range(NI):
                    ps = wo_ps.tile([128, W], F32, name="wop", tag="wop")
                    for h in range(hq):
                        mm(ps[:],
                           oTall[:, h * block + it * 128:
                                 h * block + it * 128 + 128],
                           wts[h][:], start=(h == 0), stop=(h == hq - 1))
                    ob = out_pool.tile([128, W], F32, name="ob", tag="ob")
                    nc.scalar.copy(ob[:], ps[:])
                    # stores ride the ACT ring so they never queue behind
                    # the next chunk's 16 wo-tile loads on the sync ring
                    nc.scalar.dma_start(
                        out[it * 128:(it + 1) * 128, nch * W:(nch + 1) * W],
                        ob[:])
    _trim_dma_waits(nc)
    import json as _json
    _fixed = _json.dumps(_split_waits_json(
        _json.loads(nc.to_json_bytes()))).encode()
    nc.to_json_bytes = lambda: _fixed
    return nc


def _deinterleave_cols(w, nheads):
    """Per head, reorder the 128 columns to [even head-dims, odd head-dims]."""
    dim = w.shape[0]
    r = w.reshape(dim, nheads, D // 2, 2)
    return np.concatenate([r[..., 0], r[..., 1]], axis=2).reshape(dim, nheads * D)


def _bf(a):
    return np.ascontiguousarray(a.astype(ml_dtypes.bfloat16))


def shard_inputs(x, wq, wk, wv, wo, freqs_cos, freqs_sin):
    """Build the 8 per-core input maps (core = 2*block + head_group)."""
    x = np.asarray(x, dtype=np.float32)
    wq_p = _deinterleave_cols(np.asarray(wq, dtype=np.float32), 32)
    wk_p = _deinterleave_cols(np.asarray(wk, dtype=np.float32), 8)
    wv = np.asarray(wv, dtype=np.float32)
    wo = np.asarray(wo, dtype=np.float32)
    cos = np.asarray(freqs_cos, dtype=np.float32)
    sin = np.asarray(freqs_sin, dtype=np.float32)

    wq_h = wq_p.reshape(DIM, 32, D)
    wk_h = wk_p.reshape(DIM, 8, D)
    wv_h = wv.reshape(DIM, 8, D)
    wo_h = wo.reshape(32, D, DIM)

    in_maps = []
    for core in range(N_CORES):
        b, g = divmod(core, 2)
        rows = slice(b * BLOCK, (b + 1) * BLOCK)
        cosT = cos[rows].T                       # [64, block]
        sinT = sin[rows].T
        cos2 = np.concatenate([cosT, cosT], axis=0)     # [128, block]
        sin2 = np.concatenate([-sinT, sinT], axis=0)
        in_maps.append({
            "xbT": _bf(x[rows, :].T),
            "wq": _bf(wq_h[:, g * HQ:(g + 1) * HQ].reshape(DIM, HQ * D)),
            "wk": _bf(wk_h[:, g * HKV:(g + 1) * HKV].reshape(DIM, HKV * D)),
            "wv": _bf(wv_h[:, g * HKV:(g + 1) * HKV].reshape(DIM, HKV * D)),
            "wo": _bf(wo_h[g * HQ:(g + 1) * HQ].reshape(HQ * D, DIM)),
            "cos2": _bf(cos2),
            "sin2": _bf(sin2),
        })
    return in_maps


def unshard_output(core_outs):
    full = np.empty((NB_TOTAL, DIM), dtype=np.float32)
    for b in range(NB_TOTAL // BLOCK):
        full[b * BLOCK:(b + 1) * BLOCK] = core_outs[2 * b] + core_outs[2 * b + 1]
    return full


NB_TOTAL = 4096  # total sequence length

_NC_CACHE = {}


def _get_nc():
    key = (DIM, BLOCK, HQ, HKV)
    if key not in _NC_CACHE:
        _NC_CACHE[key] = build_kernel()
    return _NC_CACHE[key]


def kernel(x, wq, wk, wv, wo, freqs_cos, freqs_sin, block_size, **run_kwargs):
    assert int(block_size) == BLOCK, f"unexpected block_size {block_size}"
    in_maps = shard_inputs(x, wq, wk, wv, wo, freqs_cos, freqs_sin)
    nc = _get_nc()
    res = bass_utils.run_bass_kernel_spmd(
        nc, in_maps, core_ids=list(range(N_CORES)), **run_kwargs)
    outs = [r["out"] for r in res.results]
    out = unshard_output(outs)
    kernel.last_results = res
    return out


# revision 38
# speedup vs baseline: 1.0104x; 1.0104x over previous
"""Block-diagonal causal GQA attention with RoPE, sharded over 8 TRN2 cores.

Problem (hardcoded from the spec):
  x [4096, 4096], wq [4096, 4096] (32 q heads x 128), wk/wv [4096, 1024]
  (8 kv heads), wo [4096, 4096], freqs_cos/sin [4096, 64], block_size 1024.
  4 independent causal blocks of 1024 tokens.

Sharding: 8 cores = 4 sequence blocks x 2 head-groups.  Core (b, g)
computes block b for q-heads [16g, 16g+16) (kv heads [4g, 4g+4)) and the
partial output projection through the matching rows of wo.  The host sums
the two head-group partials per block and concatenates the blocks.

Device layout notes:
  - all inputs are fed as bf16 (host-side cast); PSUM accumulation and the
    final output stay fp32.  End-to-end quantization error ~5e-3 rel.
  - x block is fed pre-transposed (xbT [4096, 1024]) so the DIM contraction
    sits on SBUF partitions for the QKV projections.
  - wq/wk columns are de-interleaved per head on the host (even head-dims
    then odd head-dims) so RoPE's pair rotation becomes a [0:64]/[64:128]
    partition-half operation on the Q^T/K^T layout.
  - attention runs in the transposed-score layout S^T [j, i]: scores come
    out of the PE as S^T tiles, the causal mask is accumulated into the
    diagonal blocks by a tiny PE matmul (maskT @ identity), the softmax
    denominator is a ones-vector matmul, and P^T feeds the PV and WO
    matmuls directly (no transposes).
  - the softmax reciprocal is split across engines (chunk 0: DVE
    reciprocal; chunk 1: ACT exp(-ln(x))) and its K=1 broadcast matmul +
    normalize are deferred one head, so the PE never waits on it.
  - wo tiles for the first output chunks prefetch during attention (the
    DMA rings are idle there).
"""

import numpy as np
from contextlib import ExitStack

import concourse.bass as bass
import concourse.tile as tile
import concourse.mybir as mybir
from concourse import bass_utils
import ml_dtypes

F32 = mybir.dt.float32
BF16 = mybir.dt.bfloat16
F32R = mybir.dt.float32r

# -- full-problem constants ---------------------------------------------------
DIM = 4096
BLOCK = 1024
D = 128            # head dim
HQ = 16            # q heads per core
HKV = 4            # kv heads per core
N_CORES = 8
NEG = -1.0e9


def _trim_dma_waits(nc):
    """Drop DMA semaphore waits that are transitively guaranteed.

    The DGE descriptor path supports only 2 sync-wait commands per DMA,
    but Tile's wait emission is not transitively minimal.  We compute,
    for every instruction, a conservative "floor": the semaphore values
    guaranteed to have been reached by the time it completes (its own
    waits, the floors of the instructions those waits observe, the
    floors of its sync dependencies, plus in-order completion along each
    semaphore's single FIFO ring).  A wait on a DMA is dead if the
    floors implied by its remaining waits already cover it.
    """
    import bass_rust

    insts = []
    for blk in nc.m.functions[0].blocks:
        insts.extend(blk.instructions)

    floors: dict[str, dict[int, int]] = {}     # inst name -> {sem id: value}
    chain: dict[int, list[tuple[int, str]]] = {}  # sem id -> [(post_val, name)]
    cum: dict[int, int] = {}

    def sem_floor(sem_id, v):
        """Floor implied by observing sem_id >= v (completion of the
        instruction whose update reached v, FIFO within a sem)."""
        lst = chain.get(sem_id)
        if not lst:
            return None
        # smallest post_val >= v
        import bisect
        idx = bisect.bisect_left(lst, (v, ""))
        if idx == len(lst):
            return None
        return floors.get(lst[idx][1])

    def merge(dst, src):
        if not src:
            return
        for k, v in src.items():
            if dst.get(k, -1) < v:
                dst[k] = v

    for ins in insts:
        si = ins.sync_info
        fl: dict[int, int] = {}
        if si is not None:
            for w in si.on_wait:
                if w.wait_mode != "sem-ge-imm" or w.wait_value is None:
                    continue
                if fl.get(w.id, -1) < w.wait_value:
                    fl[w.id] = w.wait_value
                merge(fl, sem_floor(w.id, w.wait_value))
        try:
            for dn in ins.sync_dependency_names():
                merge(fl, floors.get(dn))
        except TypeError:
            pass
        if si is not None:
            for u in si.on_update:
                if u.update_mode not in ("sem-add-imm", "sem-inc") \
                        or u.update_value is None:
                    continue
                post = cum.get(u.id, 0) + u.update_value
                cum[u.id] = post
                lst = chain.setdefault(u.id, [])
                # in-order completion per sem ring: inherit previous floor
                if lst:
                    merge(fl, floors.get(lst[-1][1]))
                if fl.get(u.id, -1) < post:
                    fl[u.id] = post
                lst.append((post, ins.name))
        floors[ins.name] = fl

    for ins in insts:
        if not isinstance(ins, mybir.InstDMACopy):
            continue
        si = ins.sync_info
        if si is None:
            continue
        waits = list(si.on_wait)
        changed = True
        while len(waits) > 1 and changed:
            changed = False
            for i, w in enumerate(waits):
                if w.wait_mode != "sem-ge-imm" or w.wait_value is None:
                    continue
                implied: dict[int, int] = {}
                for j, w2 in enumerate(waits):
                    if j == i or w2.wait_mode != "sem-ge-imm":
                        continue
                    merge(implied, sem_floor(w2.id, w2.wait_value))
                if implied.get(w.id, -1) >= w.wait_value:
                    waits.pop(i)
                    changed = True
                    break
        if len(waits) != len(si.on_wait):
            ins.sync_info = bass_rust.SyncInfo(
                on_wait=waits, on_update=list(si.on_update))


def _split_waits_json(bir):
    """Split multi-wait instructions at the BIR level.

    walrus' setupSyncWait budget: one wait of any value, or two waits
    whose values both fit a one-byte command.  Excess waits move onto
    standalone EventSemaphore instructions inserted directly before the
    instruction on the same engine — engines execute their stream in
    order, so a prefix wait is semantically identical to an attached
    one.  (DMAs are enqueued by their issuing engine in stream order,
    so the same argument holds for the enqueue.)
    """
    nid = 0
    for fn in bir["functions"]:
        for blk in fn["blocks"]:
            out = []
            for ins in blk["instructions"]:
                si = ins.get("sync_info")
                waits = (si or {}).get("on_wait") or []
                if len(waits) > 1:
                    waits = sorted(
                        waits, key=lambda w: -(w.get("wait_value") or 0))
                    for w in waits[1:]:
                        nid += 1
                        out.append({
                            "debug": ins.get("debug"),
                            "engine": ins["engine"],
                            "ins": [],
                            "outs": [],
                            "name": f"{ins['name']}-w{nid}",
                            "opcode": "EventSemaphore",
                            "sync_info": {"on_update": [], "on_wait": [w]},
                        })
                    si["on_wait"] = waits[:1]
                out.append(ins)
            blk["instructions"] = out
    return bir


def build_kernel(dim=DIM, block=BLOCK, hq=HQ, hkv=HKV):
    """Emit the per-core Bass program (SPMD: same program on all cores)."""
    rep = hq // hkv
    KC = dim // 128          # contraction chunks
    assert KC % 2 == 0
    KH = KC // 2             # chunks per half
    W = min(512, block)      # psum free width
    CH = block // W          # i-chunks per block
    NI = block // 128        # j-tiles per block
    assert NI <= 8
    HSET = max(1, 8 // CH)   # q heads per psum-set
    HALF_D = D // 2
    SCALE = float(1.0 / np.sqrt(D))
    assert hkv * D <= 512
    WO_PRE = 2               # wo col-chunks prefetched during attention

    nc = bass.Bass("TRN2", target_bir_lowering=False, debug=False)

    xbT = nc.dram_tensor("xbT", [dim, block], BF16, kind="ExternalInput").ap()
    wq = nc.dram_tensor("wq", [dim, hq * D], BF16, kind="ExternalInput").ap()
    wk = nc.dram_tensor("wk", [dim, hkv * D], BF16, kind="ExternalInput").ap()
    wv = nc.dram_tensor("wv", [dim, hkv * D], BF16, kind="ExternalInput").ap()
    wo = nc.dram_tensor("wo", [hq * D, dim], BF16, kind="ExternalInput").ap()
    # cos2: [cos; cos] stacked to 128 partitions; sin2: [-sin; +sin]
    cos2 = nc.dram_tensor("cos2", [D, block], BF16, kind="ExternalInput").ap()
    sin2 = nc.dram_tensor("sin2", [D, block], BF16, kind="ExternalInput").ap()
    out = nc.dram_tensor("out", [block, dim], F32, kind="ExternalOutput").ap()

    def mm(out_ap, lhsT, rhs, **kw):
        nc.tensor.matmul(out_ap, lhsT, rhs, **kw)

    # expS free-dim layout: j-tile t occupies [offs[t], offs[t] + block - 128 t)
    offs = []
    o = 0
    for t in range(NI):
        offs.append(o)
        o += block - t * 128
    EW = o

    with tile.TileContext(nc) as tc, ExitStack() as ctx:
        const = ctx.enter_context(tc.tile_pool(name="const", bufs=1))
        # memset only emits fp32 patterns: write two packed bf16 1.0s.
        # 32 columns so the denominator matmuls fill psum partitions 0..31 /
        # 32..63 with replicated sums -- every lane the reciprocal pass
        # reads is then matmul-written (no stale-psum junk lanes).
        ones = const.tile([128, 32], BF16)         # denominator lhsT
        ONES2 = float(np.frombuffer(
            np.array([0x3F803F80], dtype=np.uint32).tobytes(),
            dtype=np.float32)[0])
        nc.gpsimd.memset(ones[:].bitcast(F32), ONES2)
        # bc lhsT (parts 0/32 used): -1 so the negated approx reciprocal
        # (see recip_both) broadcasts with the right sign
        ones_row = const.tile([128, 128], F32R)
        nc.gpsimd.memset(ones_row[:].bitcast(F32), -1.0)


        acc_pool = ctx.enter_context(tc.tile_pool(name="accs", bufs=1))
        if True:
            # persistent SBUF accumulators, one big tile each, sliced per head
            qTa = acc_pool.tile([128, hq * block], BF16)     # per head: [d, i]
            kTa = acc_pool.tile([128, hkv * block], BF16)    # per kv head: [d, j]
            va = acc_pool.tile([128, NI * hkv * D], BF16)    # per j-tile: [j, hkv*D]
            oTall = acc_pool.tile([128, hq * block], BF16)   # per head: [d, i]

            # ---- QKV projections, two k-halves so SBUF holds half of xbT.
            # half 1 runs K-heads first and emits each head's RoPE right
            # after its final accumulation, so the DVE rotation work (and
            # the swap DMAs) overlap the remaining Q/V matmuls and the
            # first attention heads are ready as early as possible. ----
            with tc.tile_pool(name="ropep", bufs=3) as rp, \
                 tc.tile_pool(name="xbp", bufs=KH) as xb_pool, \
                 tc.tile_pool(name="wsp", bufs=6) as ws_pool, \
                 tc.tile_pool(name="qkvps", bufs=8, space="PSUM") as ps_pool:

                cos_sb = rp.tile([D, block], BF16, name="cos_sb", tag="cos",
                                 bufs=1)
                sin_sb = rp.tile([D, block], BF16, name="sin_sb", tag="sin",
                                 bufs=1)
                cs_loaded = False

                def acc_store(dst, ps, half):
                    if half == 0:
                        nc.scalar.copy(dst, ps)
                    else:
                        nc.vector.tensor_add(dst, dst, ps)

                def rope(base):
                    # Layout per head: partitions [0:64] = even head-dims,
                    # [64:128] = odd.  rope(x) = x*cos2 + swap(x)*sin2 where
                    # swap exchanges the halves (SBUF->SBUF DMA) and
                    # sin2 = [-sin; +sin], cos2 = [cos; cos].
                    sw = rp.tile([D, block], BF16, name="sw", tag="sw")
                    nc.sync.dma_start(sw[0:HALF_D, :], base[HALF_D:D, :])
                    nc.sync.dma_start(sw[HALF_D:D, :], base[0:HALF_D, :])
                    tmp = rp.tile([D, block], BF16, name="rtmp", tag="rtmp")
                    nc.vector.tensor_mul(tmp[:], sw[:], sin_sb[:])
                    nc.vector.tensor_mul(base, base, cos_sb[:])
                    nc.vector.tensor_add(base, base, tmp[:])

                def q_heads(half, xb, load_xb):
                    for hs in range(0, hq, HSET):
                        nh = min(HSET, hq - hs)
                        ps = [ps_pool.tile([128, CH * W], F32, name="qps",
                                           tag="ps2", bufs=4)
                              for _ in range(nh)]
                        for kk in range(KH):
                            k = half * KH + kk
                            if load_xb and hs == 0:
                                xt = xb_pool.tile([128, block], BF16,
                                                  name="xbt", tag="xb")
                                nc.sync.dma_start(
                                    xt[:], xbT[k * 128:(k + 1) * 128, :])
                                xb.append(xt)
                            wt = ws_pool.tile([128, HSET * D], BF16,
                                              name="wqs", tag="ws")
                            nc.sync.dma_start(
                                wt[:, :nh * D],
                                wq[k * 128:(k + 1) * 128, hs * D:(hs + nh) * D])
                            for hl in range(nh):
                                for c in range(CH):
                                    mm(ps[hl][:, c * W:(c + 1) * W],
                                       wt[:, hl * D:(hl + 1) * D],
                                       xb[kk][:, c * W:(c + 1) * W],
                                       start=(kk == 0), stop=(kk == KH - 1))
                        for hl in range(nh):
                            h = hs + hl
                            acc_store(qTa[:, h * block:(h + 1) * block],
                                      ps[hl][:], half)
                            if half == 1:
                                rope(qTa[:, h * block:(h + 1) * block])

                def k_heads(half, xb, load_xb):
                    for hs in range(0, hkv, HSET):
                        nh = min(HSET, hkv - hs)
                        ps = [ps_pool.tile([128, CH * W], F32, name="kps",
                                           tag="ps2", bufs=4)
                              for _ in range(nh)]
                        for kk in range(KH):
                            k = half * KH + kk
                            if load_xb and hs == 0:
                                xt = xb_pool.tile([128, block], BF16,
                                                  name="xbt", tag="xb")
                                nc.sync.dma_start(
                                    xt[:], xbT[k * 128:(k + 1) * 128, :])
                                xb.append(xt)
                            wt = ws_pool.tile([128, HSET * D], BF16,
                                              name="wks", tag="ws")
                            nc.sync.dma_start(
                                wt[:, :nh * D],
                                wk[k * 128:(k + 1) * 128, hs * D:(hs + nh) * D])
                            for hl in range(nh):
                                for c in range(CH):
                                    mm(ps[hl][:, c * W:(c + 1) * W],
                                       wt[:, hl * D:(hl + 1) * D],
                                       xb[kk][:, c * W:(c + 1) * W],
                                       start=(kk == 0), stop=(kk == KH - 1))
                        for hl in range(nh):
                            h = hs + hl
                            acc_store(kTa[:, h * block:(h + 1) * block],
                                      ps[hl][:], half)
                            if half == 1:
                                rope(kTa[:, h * block:(h + 1) * block])

                def v_heads(half, xb):
                    VP = hkv * D
                    vpair = 2 if (NI % 2 == 0 and VP * 4 >= 2048) else 1
                    for ts in range(0, NI, 8):
                        nt = min(8, NI - ts)
                        ps = [ps_pool.tile([128, vpair * VP], F32, name="vps",
                                           tag="ps2" if vpair == 2 else "ps",
                                           bufs=4)
                              for _ in range(nt // vpair)]
                        for kk in range(KH):
                            wt = ws_pool.tile([128, HSET * D], BF16,
                                              name="wvs", tag="ws")
                            k = half * KH + kk
                            nc.sync.dma_start(
                                wt[:, :VP], wv[k * 128:(k + 1) * 128, :])
                            for tl in range(nt):
                                tj = ts + tl
                                o = (tl % vpair) * VP
                                mm(ps[tl // vpair][:, o:o + VP],
                                   xb[kk][:, tj * 128:(tj + 1) * 128],
                                   wt[:, :VP],
                                   start=(kk == 0), stop=(kk == KH - 1))
                        for tl in range(0, nt, vpair):
                            tj = ts + tl
                            dst = va[:, tj * VP:(tj + vpair) * VP]
                            acc_store(dst, ps[tl // vpair][:], half)

                # half 0: Q, K, V (xb loads interleaved into the Q loop so
                # the first weight tile isn't stuck behind 8 MB of x).
                # cos/sin loads are enqueued after the first head-set's
                # loads: the DGE ring is FIFO and rope needs them late.
                xb0 = []
                q_heads(0, xb0, load_xb=True)
                if not cs_loaded:
                    nc.sync.dma_start(cos_sb[:], cos2)
                    nc.sync.dma_start(sin_sb[:], sin2)
                    cs_loaded = True
                k_heads(0, xb0, load_xb=False)
                v_heads(0, xb0)
                # half 1: K first (rope k early), then Q (rope per head),
                # then V.
                xb1 = []
                k_heads(1, xb1, load_xb=True)
                q_heads(1, xb1, load_xb=False)
                v_heads(1, xb1)

            # wo prefetch pool: opened before the attention pools so the
            # first WO_PRE column-chunks stream in while attention runs
            # (the DMA rings are otherwise idle there).
            wo_pool = ctx.enter_context(
                tc.tile_pool(name="wow", bufs=2 * hq, side="right"))
            wo_tiles = {}
            for nch in range(WO_PRE):
                wts = []
                for h in range(hq):
                    wt = wo_pool.tile([128, W], BF16, name="wot", tag="wot")
                    nc.sync.dma_start(
                        wt[:], wo[h * D:(h + 1) * D, nch * W:(nch + 1) * W])
                    wts.append(wt)
                wo_tiles[nch] = wts

            # ---- attention, 16 heads, software-pipelined ----
            # Per head h the PE stream is:
            #   bc(h-1,c0) | scores t0..t7 (+mask) with bc(h-1,c1) after t0
            #   | denominator ones-mms | PV
            # and the DVE/ACT work (exp, reciprocal, normalize) always
            # references data from >= 1 head earlier, so the PE never
            # stalls and HAM stays warm.
            # Scores psum groups pack the causal trapezoid's 8 j-tiles into
            # 5 contiguous regions -- (t0) (t1|t7|t1) (t2|t6|t2) (t3|t5|t3)
            # (t4) -- so softmax exp needs only 5 ACT instructions per head
            # (ACT's ~0.3us fixed cost per instruction made 8+4 the
            # bottleneck).  Every segment stays inside one psum bank.
            assert NI == 8 and CH == 2
            GROUPS = [[(0, 0), (0, 1)],
                      [(1, 0), (7, 1), (1, 1)],
                      [(2, 0), (6, 1), (2, 1)],
                      [(3, 0), (5, 1), (3, 1)],
                      [(4, 1)]]

            def segw(t, c):
                return (c + 1) * W - max(t * 128, c * W)

            seg_off = {}
            gbase = []
            o = 0
            for grp in GROUPS:
                gbase.append(o)
                for t, c in grp:
                    seg_off[(t, c)] = o
                    o += segw(t, c)
            assert o == EW

            with tc.tile_pool(name="attsb", bufs=2) as att_sb, \
                 tc.tile_pool(name="stps", bufs=2, space="PSUM") as st_ps, \
                 tc.tile_pool(name="st4ps", bufs=1, space="PSUM") as st4_ps, \
                 tc.tile_pool(name="sumps", bufs=1, space="PSUM") as sum_ps, \
                 tc.tile_pool(name="pvps", bufs=1, space="PSUM") as pv_ps:

                def denom_c(h, expS, sp, c):
                    # ones^T @ expS^T for chunk c; lands on psum partition
                    # 32c so both chunks share one psum bank
                    live = [t for t in range(NI) if t * 128 < (c + 1) * W]
                    for idx, t in enumerate(live):
                        s0 = max(t * 128, c * W)
                        w = (c + 1) * W - s0
                        mm(sp[32 * c:32 * c + 32, s0 - c * W:s0 - c * W + w],
                           ones[:, 0:32], expS[:, seg_off[(t, c)]:
                                               seg_off[(t, c)] + w],
                           start=(idx == 0), stop=(idx == len(live) - 1))

                # 1/den for both chunks in one 3-instruction DVE pass over
                # partitions 0..32 (lanes 1..31 compute junk that nothing
                # reads): bitwise-NOT seed + one Newton step, ~0.2% error.
                # Produces -1/den; the -1 ones_row broadcast flips it back.
                # Replaces the 3.3us iterative divide + ACT ln/exp pair.
                RC_C0, RC_C1 = -0.23549792, 2.0017324

                def recip_both(h, sp, rc):
                    # nt = ~bits(den); u = den*nt; v = c0*u - c1;
                    # rc = (v*c0)*nt = -1/den (to ~0.2%)
                    nt = att_sb.tile([64, W], F32, name="rnt", tag="rnt")
                    u = att_sb.tile([64, W], F32, name="rtu", tag="rtu")
                    nc.vector.tensor_scalar(
                        nt[:].bitcast(mybir.dt.int32), 
                        sp[0:64, :W].bitcast(mybir.dt.int32), 0, None,
                        mybir.AluOpType.bitwise_not)
                    nc.vector.tensor_mul(u[:, :W], sp[0:64, :W], nt[:, :W])
                    nc.vector.tensor_scalar(
                        u[:, :W], u[:, :W], RC_C0, RC_C1,
                        mybir.AluOpType.mult, mybir.AluOpType.subtract)
                    with nc.allow_low_precision("f32r matmul operand"):
                        nc.vector.scalar_tensor_tensor(
                            rc[0:64, :W], u[:, :W], RC_C0, nt[:, :W],
                            mybir.AluOpType.mult, mybir.AluOpType.mult)

                def pv_mms(h, expS):
                    kv = h // rep
                    pv = pv_ps.tile([128, CH * W], F32, name="pv", tag="pv")
                    for c in range(CH):
                        live = [t for t in range(NI) if t * 128 < (c + 1) * W]
                        for idx, t in enumerate(live):
                            s0 = max(t * 128, c * W)
                            w = (c + 1) * W - s0
                            mm(pv[:, s0:s0 + w],
                               va[:, t * hkv * D + kv * D:
                                  t * hkv * D + (kv + 1) * D],
                               expS[:, seg_off[(t, c)]:seg_off[(t, c)] + w],
                               start=(idx == 0), stop=(idx == len(live) - 1))
                    return pv

                def bc_norm(h, rc, pv, c):
                    # broadcast 1/den over partitions (K=1 matmul into the
                    # st4 psum bank, idle between heads), stage to SBUF on
                    # DVE (DVE reads only one PSUM operand), then normalize
                    bc = st4_ps.tile([128, W], F32, name="bc", tag="st4")
                    mm(bc[:], ones_row[32 * c:32 * c + 1, :],
                       rc[32 * c:32 * c + 1, :W], start=True, stop=True)
                    rb = att_sb.tile([128, W], F32, name="rb", tag="rb")
                    nc.vector.tensor_copy(rb[:], bc[:])
                    nc.vector.tensor_mul(
                        oTall[:, h * block + c * W:h * block + (c + 1) * W],
                        pv[:, c * W:(c + 1) * W], rb[:])

                def emit_scores(h, expS, gis):
                    kv = h // rep
                    qT = qTa[:, h * block:(h + 1) * block]
                    kT = kTa[:, kv * block:(kv + 1) * block]
                    for gi in gis:
                        grp = GROUPS[gi]
                        gw = sum(segw(t, c) for t, c in grp)
                        stt = (st_ps.tile([128, CH * W], F32, name="st",
                                          tag="st") if gi < 4 else
                               st4_ps.tile([128, W], F32, name="st4",
                                           tag="st4"))
                        for t, c in grp:
                            s0 = max(t * 128, c * W)
                            w = (c + 1) * W - s0
                            lo = seg_off[(t, c)] - gbase[gi]
                            mm(stt[:, lo:lo + w],
                               kT[:, t * 128:t * 128 + 128],
                               qT[:, s0:s0 + w], start=True, stop=True)
                        nc.scalar.activation(
                            expS[:, gbase[gi]:gbase[gi] + gw], stt[:, 0:gw],
                            mybir.ActivationFunctionType.Exp, scale=SCALE)
                        # causal mask: zero exp of the upper triangle of
                        # each j-tile's diagonal 128x128 block (keep where
                        # i_local >= j_local) on GPSIMD
                        for t, c in grp:
                            if c != (t * 128) // W:
                                continue
                            nc.gpsimd.affine_select(
                                out=expS[:, seg_off[(t, c)]:
                                         seg_off[(t, c)] + 128],
                                in_=expS[:, seg_off[(t, c)]:
                                         seg_off[(t, c)] + 128],
                                compare_op=mybir.AluOpType.is_ge,
                                fill=0.0, base=0, pattern=[[1, 128]],
                                channel_multiplier=-1,
                            )

                # Three-stage software pipeline over heads: iteration k runs
                # scores+exp of head k on PE+ACT, denominators+reciprocal+PV
                # of head k-1 (their exps completed last iteration), and the
                # broadcast+normalize of head k-2.  The head k-1/k-2 work is
                # interleaved BETWEEN head k's score groups so each st psum
                # slot gets ~1.5us of slack before its reuse and the PE
                # stream never drains (draining also re-throttles HAM).
                S = {}          # live per-head state: expS/sp/rc/pv
                for k in range(hq + 1):
                    live = k < hq
                    if live:
                        S[k] = {"expS": att_sb.tile([128, EW], BF16,
                                                    name="expS", tag="expS"),
                                "rc": att_sb.tile([64, W], F32R,
                                                  name="rc", tag="rc")}
                        emit_scores(k, S[k]["expS"], [0])
                    if k >= 2:
                        bc_norm(k - 2, S[k - 2]["rc"], S[k - 2]["pv"], 0)
                    if live:
                        emit_scores(k, S[k]["expS"], [1])
                    if k >= 1:
                        S[k - 1]["sp"] = sum_ps.tile([128, W], F32,
                                                     name="sump", tag="sump")
                        denom_c(k - 1, S[k - 1]["expS"], S[k - 1]["sp"], 0)
                    if live:
                        emit_scores(k, S[k]["expS"], [2])
                    if k >= 2:
                        bc_norm(k - 2, S[k - 2]["rc"], S[k - 2]["pv"], 1)
                        del S[k - 2]
                    if live:
                        emit_scores(k, S[k]["expS"], [3])
                    if k >= 1:
                        denom_c(k - 1, S[k - 1]["expS"], S[k - 1]["sp"], 1)
                        recip_both(k - 1, S[k - 1]["sp"], S[k - 1]["rc"])
                    if live:
                        emit_scores(k, S[k]["expS"], [4])
                    if k >= 1:
                        S[k - 1]["pv"] = pv_mms(k - 1, S[k - 1]["expS"])
                bc_norm(hq - 1, S[hq - 1]["rc"], S[hq - 1]["pv"], 0)
                bc_norm(hq - 1, S[hq - 1]["rc"], S[hq - 1]["pv"], 1)

        # ---- output projection: out = O @ wo_g ----
        with tc.tile_pool(name="woout", bufs=4) as out_pool, \
             tc.tile_pool(name="wops", bufs=4, space="PSUM") as wo_ps:
            for nch in range(dim // W):
                if nch in wo_tiles:
                    wts = wo_tiles[nch]
                else:
                    wts = []
                    for h in range(hq):
                        wt = wo_pool.tile([128, W], BF16, name="wot",
                                          tag="wot")
                        nc.sync.dma_start(
                            wt[:],
                            wo[h * D:(h + 1) * D, nch * W:(nch + 1) * W])
                        wts.append(wt)
                for it in range(NI):
                    ps = wo_ps.tile([128, W], F32, name="wop", tag="wop")
                    for h in range(hq):
                        mm(ps[:],
                           oTall[:, h * block + it * 128:
                                 h * block + it * 128 + 128],
                           wts[h][:], start=(h == 0), stop=(h == hq - 1))
                    ob = out_pool.tile([128, W], F32, name="ob", tag="ob")
                    nc.scalar.copy(ob[:], ps[:])
                    nc.sync.dma_start(
                        out[it * 128:(it + 1) * 128, nch * W:(nch + 1) * W],
                        ob[:])
    _trim_dma_waits(nc)
    import json as _json
    _fixed = _json.dumps(_split_waits_json(
        _json.loads(nc.to_json_bytes()))).encode()
    nc.to_json_bytes = lambda: _fixed
    return nc


def _deinterleave_cols(w, nheads):
    """Per head, reorder the 128 columns to [even head-dims, odd head-dims]."""
    dim = w.shape[0]
    r = w.reshape(dim, nheads, D // 2, 2)
    return np.concatenate([r[..., 0], r[..., 1]], axis=2).reshape(dim, nheads * D)


def _bf(a):
    return np.ascontiguousarray(a.astype(ml_dtypes.bfloat16))


def shard_inputs(x, wq, wk, wv, wo, freqs_cos, freqs_sin):
    """Build the 8 per-core input maps (core = 2*block + head_group)."""
    x = np.asarray(x, dtype=np.float32)
    wq_p = _deinterleave_cols(np.asarray(wq, dtype=np.float32), 32)
    wk_p = _deinterleave_cols(np.asarray(wk, dtype=np.float32), 8)
    wv = np.asarray(wv, dtype=np.float32)
    wo = np.asarray(wo, dtype=np.float32)
    cos = np.asarray(freqs_cos, dtype=np.float32)
    sin = np.asarray(freqs_sin, dtype=np.float32)

    wq_h = wq_p.reshape(DIM, 32, D)
    wk_h = wk_p.reshape(DIM, 8, D)
    wv_h = wv.reshape(DIM, 8, D)
    wo_h = wo.reshape(32, D, DIM)

    in_maps = []
    for core in range(N_CORES):
        b, g = divmod(core, 2)
        rows = slice(b * BLOCK, (b + 1) * BLOCK)
        cosT = cos[rows].T                       # [64, block]
        sinT = sin[rows].T
        cos2 = np.concatenate([cosT, cosT], axis=0)     # [128, block]
        sin2 = np.concatenate([-sinT, sinT], axis=0)
        in_maps.append({
            "xbT": _bf(x[rows, :].T),
            "wq": _bf(wq_h[:, g * HQ:(g + 1) * HQ].reshape(DIM, HQ * D)),
            "wk": _bf(wk_h[:, g * HKV:(g + 1) * HKV].reshape(DIM, HKV * D)),
            "wv": _bf(wv_h[:, g * HKV:(g + 1) * HKV].reshape(DIM, HKV * D)),
            "wo": _bf(wo_h[g * HQ:(g + 1) * HQ].reshape(HQ * D, DIM)),
            "cos2": _bf(cos2),
            "sin2": _bf(sin2),
        })
    return in_maps


def unshard_output(core_outs):
    full = np.empty((NB_TOTAL, DIM), dtype=np.float32)
    for b in range(NB_TOTAL // BLOCK):
        full[b * BLOCK:(b + 1) * BLOCK] = core_outs[2 * b] + core_outs[2 * b + 1]
    return full


NB_TOTAL = 4096  # total sequence length

_NC_CACHE = {}


def _get_nc():
    key = (DIM, BLOCK, HQ, HKV)
    if key not in _NC_CACHE:
        _NC_CACHE[key] = build_kernel()
    return _NC_CACHE[key]


def kernel(x, wq, wk, wv, wo, freqs_cos, freqs_sin, block_size, **run_kwargs):
    assert int(block_size) == BLOCK, f"unexpected block_size {block_size}"
    in_maps = shard_inputs(x, wq, wk, wv, wo, freqs_cos, freqs_sin)
    nc = _get_nc()
    res = bass_utils.run_bass_kernel_spmd(
        nc, in_maps, core_ids=list(range(N_CORES)), **run_kwargs)
    outs = [r["out"] for r in res.results]
    out = unshard_output(outs)
    kernel.last_results = res
    return out
